# revision 8
# baseline (speedup 1.0000x reference)
"""Trainium2 Bass kernel for nn_LiquidGenerator.

score = sum over (i, image j) pairs of (CUTOFF - dist)^2 where dist < CUTOFF,
with dist over the [N, 27N] supercell distance matrix.

Strategy (v2)
-------------
Host (O(N log N) prep):
  * generate P (rotation+translation of molecule-local coords, float64)
  * z-sort atoms; rows are processed as 8 chunks of 128 = consecutive z-slabs.
  * shift symmetry d(i,(k,j)) == d(j,(-k,i)): only 13 of the 26 non-central
    images are computed, with weight 2.
  * central pair symmetry d(i,j)==d(j,i): for row-chunk r only columns j in
    HIGHER chunks are computed (weight 2) plus the full diagonal block
    (weight 1, both orderings).
  * z-band pruning: a column (central atom or shifted image at z') can only
    interact with chunk r if [z'-3, z'+3] overlaps the chunk's z-slab. This
    cuts the per-chunk column count ~4x vs. computing every column against
    every chunk.
  * distances via the 5-feature inner product
      d^2 + BIAS = [Px,Py,Pz,|P|^2,1] . [-2Sx,-2Sy,-2Sz, 1, |S|^2+BIAS]
    (coordinates centered at the cell midpoint; BIAS keeps the PE sum > 0).

Device (8 NeuronCores; every block's columns are sharded core k <- cols k::8):
  per iteration, 4 PSUM banks (PE row group g <-> bank g), each holding two
  chunks' blocks: [diagA(16) diagB(16) w2A w2B pad->W2]. 16 self-weight-
  loading fp32 matmuls (K=5) run 4-way concurrent across row groups. Then,
  with s = sqrt(d^2+BIAS) (ScalarE, one pass over everything) and the
  identity
      (3 - min(s,3))^2 = (min(d^2,9) - 9) - 6*(min(s,3) - 3),
  whose two terms are EXACTLY zero for non-contributing pairs (no large-sum
  cancellation, no dependence on sqrt(9.0) rounding):
    ScalarE : acc0 = sum relu(9 - d^2)            over diag  (= -sum(u-9))
    VectorE : acc1 = sum (min(d^2,9) - 9)         over w2    (PSUM, 1x)
    VectorE : acc2 = sum (min(s,3) - 3)           over diag  (bf16 s, 4x)
    VectorE : acc3 = sum (min(s,3) - 3)           over w2    (bf16 s, 4x)
  score = sum_cores_partitions [-acc0 - 6 acc2 + 2 (acc1 - 6 acc3)]
          - N (3-sqrt(BIAS))^2 + N (3-sqrt(EPS))^2   (exact self-pair swap)
Loop timing: the For_i body holds `reps` back-to-back iterations so
consecutive ones pipeline through double-buffered PSUM/SBUF tiles and the
all-engine loop barrier amortizes.
"""

import numpy as np

CUTOFF = 3.0
EPS = 1e-16
BIAS = 2e-4
BAND_MARGIN = 1e-3

NCORES = 8
N = 1024
NCHUNK = 8
G = 4                # PE row groups == PSUM banks per iteration
RHS0 = 256           # rhs feature columns start after the two lhs blocks

_cache: dict = {}


# ----------------------------------------------------------------- host math
def _rotation_matrices(rot):
    a, b, g = rot[:, 0], rot[:, 1], rot[:, 2]
    ca, sa = np.cos(a), np.sin(a)
    cb, sb = np.cos(b), np.sin(b)
    cg, sg = np.cos(g), np.sin(g)
    m = rot.shape[0]
    rx = np.zeros((m, 3, 3)); ry = np.zeros((m, 3, 3)); rz = np.zeros((m, 3, 3))
    rx[:, 0, 0] = 1;  rx[:, 1, 1] = ca; rx[:, 1, 2] = -sa; rx[:, 2, 1] = sa; rx[:, 2, 2] = ca
    ry[:, 0, 0] = cb; ry[:, 0, 2] = -sb; ry[:, 1, 1] = 1;  ry[:, 2, 0] = sb; ry[:, 2, 2] = cb
    rz[:, 0, 0] = cg; rz[:, 0, 1] = -sg; rz[:, 1, 0] = sg; rz[:, 1, 1] = cg; rz[:, 2, 2] = 1
    return np.einsum("mij,mjk,mkl->mil", rx, ry, rz)


def _generate(positions, translation, rotation, cell):
    R = _rotation_matrices(rotation.astype(np.float64))
    trans = np.remainder(translation.astype(np.float64), 1.0) @ cell.astype(np.float64)
    gen = np.einsum("mai,mij->maj", positions.astype(np.float64), R) + trans[:, None, :]
    return gen.reshape(-1, 3)


def _features(S, c, bias):
    """rhs feature columns for image positions S (pairs with lhs features)."""
    Sc = (S - c).astype(np.float32)
    return np.stack([
        -2.0 * Sc[:, 0], -2.0 * Sc[:, 1], -2.0 * Sc[:, 2],
        np.ones(S.shape[0], np.float32),
        (Sc.astype(np.float64) ** 2).sum(1).astype(np.float32) + np.float32(bias),
    ]).astype(np.float32)


# ------------------------------------------------------------- bass program
def _build_program(widths, reps: int = 1, loop_n: int = 0):
    # widths: ((w2A_0, w2B_0), ..., (w2A_3, w2B_3)) per-core column widths of
    # the two weight-2 blocks in each PSUM bank; w2A+w2B == W2 for all banks.
    key = ("nc", widths, reps, loop_n)
    if key in _cache:
        return _cache[key]
    from contextlib import ExitStack, nullcontext
    import concourse.tile as tile
    from concourse import bacc, mybir

    f32 = mybir.dt.float32
    bf16 = mybir.dt.bfloat16
    W2 = widths[0][0] + widths[0][1]
    BW = 32 + W2
    FW = RHS0 + BW + 64 + 1  # last column: 9.0 bias operand for the ACT relu

    nc = bacc.Bacc("TRN2", target_bir_lowering=False, debug=False,
                   num_devices=NCORES)
    feat_d = nc.dram_tensor("feat", [128, FW], f32, kind="ExternalInput")
    acc_d = nc.dram_tensor("acc", [128, 4], f32, kind="ExternalOutput")

    with tile.TileContext(nc) as tc, ExitStack() as ctx:
        const = ctx.enter_context(tc.tile_pool(name="const", bufs=1))
        psum = ctx.enter_context(tc.tile_pool(name="psum", bufs=2, space="PSUM"))
        spool = ctx.enter_context(tc.tile_pool(name="s", bufs=2))
        scrap = ctx.enter_context(tc.tile_pool(name="scrap", bufs=2))

        ft = const.tile([128, FW], f32)
        nc.sync.dma_start(ft[:], feat_d[:])
        at = const.tile([128, 4], f32)

        # bf16-zero views of the zero-padded feat tail for "toucher" matmuls
        bw = ft[0:1, FW - 65:FW - 1].bitcast(bf16)  # [1,128]
        bx = bw[:, 0:1]
        nine = ft[:, FW - 1:FW]                     # [128,1] column of 9.0

        loop_cm = tc.For_i(0, loop_n, 1) if loop_n else nullcontext()
        with loop_cm:
            for _u in range(reps):
                ps = psum.tile([128, G * 512], f32)
                for g in range(G):
                    fl = ft[32 * g:32 * g + 5, :]
                    base = g * 512
                    wa, wb = widths[g]
                    # chunk A: diag block then its w2 block (same weights)
                    nc.tensor.matmul(
                        ps[:, base:base + 16], fl[:, 0:128],
                        fl[:, RHS0:RHS0 + 16],
                        start=True, stop=True, tile_position=(32 * g, 0))
                    nc.tensor.matmul(
                        ps[:, base + 32:base + 32 + wa], fl[:, 0:128],
                        fl[:, RHS0 + 32:RHS0 + 32 + wa],
                        start=True, stop=True, tile_position=(32 * g, 0))
                    # chunk B
                    nc.tensor.matmul(
                        ps[:, base + 16:base + 32], fl[:, 128:256],
                        fl[:, RHS0 + 16:RHS0 + 32],
                        start=True, stop=True, tile_position=(32 * g, 0))
                    nc.tensor.matmul(
                        ps[:, base + 32 + wa:base + 32 + W2], fl[:, 128:256],
                        fl[:, RHS0 + 32 + wa:RHS0 + 32 + W2],
                        start=True, stop=True, tile_position=(32 * g, 0))

                p3 = ps[:].rearrange("p (b w) -> p b w", b=G)    # [128,4,512]
                st = spool.tile([128, G * BW], bf16)
                s3 = st[:].rearrange("p (b w) -> p b w", b=G)    # [128,4,BW]
                ja = scrap.tile([128, G * W2], f32, tag="ja")
                a3 = ja[:].rearrange("p (b w) -> p b w", b=G)
                jr = scrap.tile([128, G * 32], f32, tag="jr")
                r3 = jr[:].rearrange("p (b w) -> p b w", b=G)
                jb = scrap.tile([128, G * BW], bf16, tag="jb")
                b3 = jb[:].rearrange("p (b w) -> p b w", b=G)

                # ScalarE: s = sqrt(d^2+BIAS), everything in one pass
                nc.scalar.activation(s3[:, :, 0:BW], p3[:, :, 0:BW],
                                     mybir.ActivationFunctionType.Sqrt)
                # ScalarE: acc0 = sum relu(9 - d^2) over diag
                nc.scalar.activation(r3[:, :, 0:32], p3[:, :, 0:32],
                                     mybir.ActivationFunctionType.Relu,
                                     bias=nine, scale=-1.0,
                                     accum_out=at[:, 0:1])
                # VectorE: acc1 = sum min(d^2,9) over w2 (PSUM source, 1x);
                # with accum_out, op1 is the REDUCE op (out = op0 only) and
                # the host subtracts the 9*E / 3*E offsets exactly.
                nc.vector.tensor_scalar(
                    a3[:, :, 0:W2], p3[:, :, 32:32 + W2], 9.0, None,
                    mybir.AluOpType.min, mybir.AluOpType.add,
                    accum_out=at[:, 1:2])
                # VectorE: acc2/acc3 = sum min(s,3) over diag / w2
                nc.vector.tensor_scalar(
                    b3[:, :, 0:32], s3[:, :, 0:32], CUTOFF, None,
                    mybir.AluOpType.min, mybir.AluOpType.add,
                    accum_out=at[:, 2:3])
                nc.vector.tensor_scalar(
                    b3[:, :, 32:BW], s3[:, :, 32:BW], CUTOFF, None,
                    mybir.AluOpType.min, mybir.AluOpType.add,
                    accum_out=at[:, 3:4])

                # Touchers: retake PSUM ownership on the PE so the next
                # iteration's matmuls only see same-engine deps.
                # toucher1 overlaps the ACT-read region (col 0), toucher2 the
                # DVE-read region (col 32); ACT dep of toucher2 is satisfied
                # transitively through toucher1 (same PE program order).
                nc.tensor.matmul(ps[:, 0:1], bw, bx, start=True, stop=True)
                nc.tensor.matmul(ps[:, 32:33], bw, bx, start=True, stop=True)
        nc.sync.dma_start(acc_d[:], at[:])

    nc.finalize()
    _cache[key] = nc
    return nc


# --------------------------------------------------------------- input prep
def _prepare_inputs(positions, translation, rotation, cell):
    cell64 = cell.astype(np.float64)
    P = _generate(positions, translation, rotation, cell64)      # [N,3] float64
    n = P.shape[0]
    assert n == N, f"kernel hardcodes N={N}, got {n}"

    order = np.argsort(P[:, 2], kind="stable")
    Ps = P[order]
    zs = Ps[:, 2]
    slab_lo = zs.reshape(NCHUNK, 128).min(1)
    slab_hi = zs.reshape(NCHUNK, 128).max(1)

    shifts = np.array([-1.0, 0.0, 1.0])
    offs = np.stack(np.meshgrid(shifts, shifts, shifts, indexing="ij")).reshape(3, -1).T
    vecs = offs @ cell64                                          # [27,3]
    assert np.all(offs[13] == 0.0)
    half = list(range(13))

    c = 0.5 * cell64.sum(axis=0)
    reach = CUTOFF + BAND_MARGIN
    lo = P.min(axis=0) - reach
    hi = P.max(axis=0) + reach

    # per-chunk w2 column positions (float64), before sharding
    w2_pos = []
    for r in range(NCHUNK):
        cols = []
        jlo = 128 * (r + 1)
        band = np.nonzero(zs[jlo:] <= slab_hi[r] + reach)[0]
        if band.size:
            cols.append(Ps[jlo + band])
        for k in half:
            S = Ps + vecs[k]
            m = np.all((S > lo) & (S < hi), axis=1)
            m &= (S[:, 2] >= slab_lo[r] - reach) & (S[:, 2] <= slab_hi[r] + reach)
            if m.any():
                cols.append(S[m])
        w2_pos.append(np.concatenate(cols, axis=0) if cols
                      else np.zeros((0, 3)))

    # per-core width of each chunk's w2 block, bank pairing big-with-small
    w2w = np.array([-(-len(p) // NCORES) for p in w2_pos])        # ceil/8
    idx = np.argsort(w2w, kind="stable")[::-1]
    pairs = [(int(idx[b]), int(idx[7 - b])) for b in range(G)]
    W2 = int(max(w2w[a] + w2w[b] for a, b in pairs))
    W2 = -(-W2 // 4) * 4                                          # multiple of 4
    widths = tuple((int(w2w[a]), int(W2 - w2w[a])) for a, b in pairs)

    dummy_pos = c + 50.0                                          # d^2 >> 9

    def block_cols(pos_list, core, width):
        """core's columns of a block: pos_list[core::8] padded to width."""
        sel = pos_list[core::NCORES]
        pad = width - len(sel)
        assert pad >= 0
        if pad:
            sel = np.concatenate([sel, np.tile(dummy_pos, (pad, 1))], axis=0)
        return sel

    in_maps = []
    for core in range(NCORES):
        feat = np.zeros((128, RHS0 + 32 + W2 + 64 + 1), np.float32)
        feat[:, -1] = 9.0                         # ACT relu bias column
        for g, (ca, cb) in enumerate(pairs):
            wa, wb = widths[g]
            rows = slice(32 * g, 32 * g + 5)
            # lhs features: chunk A then chunk B atoms (sorted order)
            feat[rows, 0:128] = _featT(Ps[128 * ca:128 * (ca + 1)], c)
            feat[rows, 128:256] = _featT(Ps[128 * cb:128 * (cb + 1)], c)
            # rhs: diagA diagB w2A w2B(padded)
            dA = Ps[128 * ca:128 * (ca + 1)][core::NCORES]
            dB = Ps[128 * cb:128 * (cb + 1)][core::NCORES]
            feat[rows, RHS0:RHS0 + 16] = _features(dA, c, BIAS)
            feat[rows, RHS0 + 16:RHS0 + 32] = _features(dB, c, BIAS)
            feat[rows, RHS0 + 32:RHS0 + 32 + wa] = _features(
                block_cols(w2_pos[ca], core, wa), c, BIAS)
            feat[rows, RHS0 + 32 + wa:RHS0 + 32 + W2] = _features(
                block_cols(w2_pos[cb], core, wb), c, BIAS)
        in_maps.append({"feat": np.ascontiguousarray(feat)})
    return in_maps, widths


def _featT(Patoms, c):
    """lhs feature rows [5, n] for row atoms."""
    Pc = (Patoms - c).astype(np.float32)
    return np.stack([
        Pc[:, 0], Pc[:, 1], Pc[:, 2],
        (Pc.astype(np.float64) ** 2).sum(1).astype(np.float32),
        np.ones(Patoms.shape[0], np.float32),
    ]).astype(np.float32)


# ------------------------------------------------------------------- runner
def _get_runner(widths, reps: int = 1, loop_n: int = 0):
    """Jit the bass program once; reuse the compiled executable per call."""
    key = ("runner", widths, reps, loop_n)
    if key in _cache:
        return _cache[key]
    import jax
    from jax.sharding import Mesh, PartitionSpec
    from jax.experimental.shard_map import shard_map
    from concourse import bass2jax, mybir

    nc = _build_program(widths, reps=reps, loop_n=loop_n)
    bass2jax.install_neuronx_cc_hook()

    partition_name = (
        nc.partition_id_tensor.name if nc.partition_id_tensor else None
    )
    in_names, out_names, out_avals, zero_outs = [], [], [], []
    for alloc in nc.m.functions[0].allocations:
        if not isinstance(alloc, mybir.MemoryLocationSet):
            continue
        name = alloc.memorylocations[0].name
        if alloc.kind == "ExternalInput":
            if name != partition_name:
                in_names.append(name)
        elif alloc.kind == "ExternalOutput":
            out_names.append(name)
            shape = tuple(alloc.tensor_shape)
            dtype = mybir.dt.np(alloc.dtype)
            out_avals.append(jax.core.ShapedArray(shape, dtype))
            zero_outs.append(np.zeros(shape, dtype))
    n_params = len(in_names)
    all_in_names = in_names + out_names
    if partition_name is not None:
        all_in_names = all_in_names + [partition_name]

    def _body(*args):
        operands = list(args)
        if partition_name is not None:
            operands.append(bass2jax.partition_id_tensor())
        outs = bass2jax._bass_exec_p.bind(
            *operands,
            out_avals=tuple(out_avals),
            in_names=tuple(all_in_names),
            out_names=tuple(out_names),
            lowering_input_output_aliases=(),
            sim_require_finite=True,
            sim_require_nnan=True,
            nc=nc,
        )
        return tuple(outs)

    devices = jax.devices()[:NCORES]
    mesh = Mesh(np.asarray(devices), ("core",))
    n_outs = len(out_names)
    sharded = jax.jit(
        shard_map(
            _body, mesh=mesh,
            in_specs=(PartitionSpec("core"),) * (n_params + n_outs),
            out_specs=(PartitionSpec("core"),) * n_outs,
            check_rep=False,
        ),
        keep_unused=True,
    )
    concat_zeros = [
        np.zeros((NCORES * z.shape[0], *z.shape[1:]), z.dtype) for z in zero_outs
    ]

    def run(in_maps):
        concat_in = [
            np.concatenate([in_maps[cc][name] for cc in range(NCORES)], axis=0)
            for name in in_names
        ]
        out_arrs = sharded(*concat_in, *concat_zeros)
        return [
            {
                name: np.asarray(out_arrs[i]).reshape(NCORES, *out_avals[i].shape)[cc]
                for i, name in enumerate(out_names)
            }
            for cc in range(NCORES)
        ]

    _cache[key] = run
    return run


def kernel(positions, translation, rotation, cell, _reps=1, _loop_n=0):
    in_maps, widths = _prepare_inputs(
        np.asarray(positions), np.asarray(translation),
        np.asarray(rotation), np.asarray(cell),
    )
    run = _get_runner(widths, reps=_reps, loop_n=_loop_n)
    results = run(in_maps)
    W2 = widths[0][0] + widths[0][1]
    e_diag = 128.0 * 4 * 32          # elements per core, diag region
    e_w2 = 128.0 * 4 * W2            # elements per core, w2 region
    total = 0.0
    for r in results:
        acc = r["acc"].astype(np.float64)
        # contrib = sum(u-9) - 6*sum(min(s,3)-3); diag w=1, w2 w=2
        su9_w2 = acc[:, 1].sum() - 9.0 * e_w2
        sv_diag = acc[:, 2].sum() - 3.0 * e_diag
        sv_w2 = acc[:, 3].sum() - 3.0 * e_w2
        total += (-acc[:, 0].sum() - 6.0 * sv_diag
                  + 2.0 * (su9_w2 - 6.0 * sv_w2))
    # swap device self-pair terms for the exact ones
    total -= N * (CUTOFF - np.sqrt(BIAS)) ** 2
    total += N * (CUTOFF - np.sqrt(np.float32(EPS))) ** 2
    return np.float32(total)


# revision 11
# speedup vs baseline: 2.2213x; 2.2213x over previous
"""Trainium2 Bass kernel for nn_LiquidGenerator.

score = sum over (i, image j) pairs of (CUTOFF - dist)^2 where dist < CUTOFF,
with dist over the [N, 27N] supercell distance matrix.

Strategy (v3)
-------------
Host (O(N log N) prep):
  * generate P (rotation+translation of molecule-local coords, float64)
  * z-sort atoms; rows are processed as 8 chunks of 128 = consecutive z-slabs.
  * central pair symmetry d(i,j)==d(j,i): for row-chunk r only columns j in
    HIGHER chunks are computed (weight 2) plus the full diagonal block
    (weight 1, both orderings).
  * shift symmetry d(i,(k,j)) == d(j,(26-k,i)): one member of each of the 13
    image pairs is computed with weight 2; WHICH member is chosen greedily to
    flatten the per-chunk column loads (the two choices land on mirrored z
    ranges).
  * z-band pruning: a column (central atom or image at z') only pairs with
    chunk r if [z'-3, z'+3] overlaps the chunk's z-slab (~4x fewer elements).
  * distances via the 5-feature inner product
      d^2 + BIAS = [Px,Py,Pz,|P|^2,1] . [-2Sx,-2Sy,-2Sz, 1, |S|^2+BIAS].

Device (8 NeuronCores; every block's columns are sharded core k <- cols k::8):
  per iteration one 4-bank PSUM tile holds 8 uniform units [diag(16)|w2(WM)],
  two per bank: unit = one chunk's diag + weight-2 columns, one self-loading
  fp32 matmul each (8 matmuls, 4-way row-group concurrency).  The weight-2
  factor is folded into the VALUES, not the accumulation:
      sqrt-w2 pass uses scale=2:  s~ = sqrt(2(d^2+B)) = sqrt2 * s
      v' = min(s~, 3*sqrt2) - 3*sqrt2 = sqrt2 * (min(s,3)-3)
  so v'^2 = 2 v^2 and ONE scalar_tensor_tensor square-accumulate over the
  whole tile yields sum(v_diag^2) + 2 sum(v_w2^2) in a single accumulator
  (one DVE accumulator-read per iteration).  All terms are exactly zero for
  non-contributing pairs: no big-sum cancellation, sqrt-spline-safe.
    ScalarE : s~ = sqrt(2(d^2+B)) over w2, s = sqrt(d^2+B) over diag
    VectorE : v' = min(s,3)-3 / min(s~,3sqrt2)-3sqrt2   (bf16, 4x mode)
    VectorE : acc += v'*v' (scalar_tensor_tensor, 2x mode, accum_out)
  score = sum acc - N (3-sqrt(BIAS))^2 + N (3-sqrt(EPS))^2

The timing loop uses a DYNAMIC trip count (read from the `loopn` input) so
one compiled program serves every loop length: the PJRT dispatch constant
cancels exactly in the (wall(N) - wall(1)) / (N-1) slope.  The body holds
`reps` back-to-back iterations so consecutive ones pipeline through the
double-buffered PSUM/SBUF tiles and the all-engine barrier amortizes.
"""

import numpy as np

CUTOFF = 3.0
EPS = 1e-16
BIAS = 2e-4
BAND_MARGIN = 1e-3

NCORES = 8
N = 1024
NCHUNK = 8
G = 4                # PE row groups == PSUM banks
RHS0 = 256           # rhs feature columns start after the two lhs blocks

_cache: dict = {}


# ----------------------------------------------------------------- host math
def _rotation_matrices(rot):
    a, b, g = rot[:, 0], rot[:, 1], rot[:, 2]
    ca, sa = np.cos(a), np.sin(a)
    cb, sb = np.cos(b), np.sin(b)
    cg, sg = np.cos(g), np.sin(g)
    m = rot.shape[0]
    rx = np.zeros((m, 3, 3)); ry = np.zeros((m, 3, 3)); rz = np.zeros((m, 3, 3))
    rx[:, 0, 0] = 1;  rx[:, 1, 1] = ca; rx[:, 1, 2] = -sa; rx[:, 2, 1] = sa; rx[:, 2, 2] = ca
    ry[:, 0, 0] = cb; ry[:, 0, 2] = -sb; ry[:, 1, 1] = 1;  ry[:, 2, 0] = sb; ry[:, 2, 2] = cb
    rz[:, 0, 0] = cg; rz[:, 0, 1] = -sg; rz[:, 1, 0] = sg; rz[:, 1, 1] = cg; rz[:, 2, 2] = 1
    return np.einsum("mij,mjk,mkl->mil", rx, ry, rz)


def _generate(positions, translation, rotation, cell):
    R = _rotation_matrices(rotation.astype(np.float64))
    trans = np.remainder(translation.astype(np.float64), 1.0) @ cell.astype(np.float64)
    gen = np.einsum("mai,mij->maj", positions.astype(np.float64), R) + trans[:, None, :]
    return gen.reshape(-1, 3)


def _features(S, c, bias):
    """rhs feature columns for image positions S (pairs with lhs features)."""
    Sc = (S - c).astype(np.float32)
    return np.stack([
        -2.0 * Sc[:, 0], -2.0 * Sc[:, 1], -2.0 * Sc[:, 2],
        np.ones(S.shape[0], np.float32),
        (Sc.astype(np.float64) ** 2).sum(1).astype(np.float32) + np.float32(bias),
    ]).astype(np.float32)


def _featT(Patoms, c):
    """lhs feature rows [5, n] for row atoms."""
    Pc = (Patoms - c).astype(np.float32)
    return np.stack([
        Pc[:, 0], Pc[:, 1], Pc[:, 2],
        (Pc.astype(np.float64) ** 2).sum(1).astype(np.float32),
        np.ones(Patoms.shape[0], np.float32),
    ]).astype(np.float32)


# ------------------------------------------------------------- bass program
def _build_program(wm: int, reps: int = 1, dyn_loop: bool = False):
    # wm: uniform per-core w2 width of each unit; unit width UW = 16 + wm.
    key = ("nc", wm, reps, dyn_loop)
    if key in _cache:
        return _cache[key]
    from contextlib import ExitStack, nullcontext
    import concourse.tile as tile
    from concourse import bacc, mybir

    f32 = mybir.dt.float32
    bf16 = mybir.dt.bfloat16
    i32 = mybir.dt.int32
    UW = 16 + wm
    FD = 2 * G * UW                   # elements per partition per iteration
    FW = RHS0 + 2 * UW + 64           # lhsA lhsB | rhsA rhsB | zero tail
    T2 = float(np.float32(3.0 * np.sqrt(2.0)))

    nc = bacc.Bacc("TRN2", target_bir_lowering=False, debug=False,
                   num_devices=NCORES)
    feat_d = nc.dram_tensor("feat", [128, FW], f32, kind="ExternalInput")
    if dyn_loop:
        loopn_d = nc.dram_tensor("loopn", [1, 1], i32, kind="ExternalInput")
    acc_d = nc.dram_tensor("acc", [128, 1], f32, kind="ExternalOutput")

    with tile.TileContext(nc) as tc, ExitStack() as ctx:
        const = ctx.enter_context(tc.tile_pool(name="const", bufs=1))
        psum = ctx.enter_context(tc.tile_pool(name="psum", bufs=2, space="PSUM"))
        spool = ctx.enter_context(tc.tile_pool(name="s", bufs=2))
        vpool = ctx.enter_context(tc.tile_pool(name="v", bufs=2))
        qpool = ctx.enter_context(tc.tile_pool(name="q", bufs=2))

        ft = const.tile([128, FW], f32)
        nc.sync.dma_start(ft[:], feat_d[:])
        at = const.tile([128, 1], f32)

        # bf16-zero views of the zero-padded feat tail for the toucher matmul
        bw = ft[0:1, FW - 64:FW].bitcast(bf16)  # [1,128]
        bx = bw[:, 0:1]

        if dyn_loop:
            lt = const.tile([1, 1], i32)
            nc.sync.dma_start(lt[:], loopn_d[:])
            nval = nc.values_load(lt[0:1, 0:1], min_val=1, max_val=1 << 30,
                                  skip_runtime_bounds_check=True)
            loop_cm = tc.For_i(0, nval, 1)
        else:
            loop_cm = nullcontext()
        with loop_cm:
            for _u in range(reps):
                ps = psum.tile([128, G * 512], f32)
                for g in range(G):
                    fl = ft[32 * g:32 * g + 5, :]
                    for u in range(2):
                        # one matmul per chunk: [diag(16) | w2(wm)] unit
                        nc.tensor.matmul(
                            ps[:, g * 512 + u * UW:g * 512 + (u + 1) * UW],
                            fl[:, 128 * u:128 * (u + 1)],
                            fl[:, RHS0 + u * UW:RHS0 + (u + 1) * UW],
                            start=True, stop=True, tile_position=(32 * g, 0))

                # 4D views: [partition, bank, unit, col]
                p4 = ps[:].rearrange("p (b w) -> p b w", b=G)[:, :, 0:2 * UW] \
                    .rearrange("p b (u w) -> p b u w", u=2)
                st = spool.tile([128, 2 * G * UW], bf16)
                s4 = st[:].rearrange("p (b u w) -> p b u w", b=G, u=2)
                jv = vpool.tile([128, 2 * G * UW], bf16)
                v4 = jv[:].rearrange("p (b u w) -> p b u w", b=G, u=2)
                jq = qpool.tile([128, 2 * G * UW], bf16)

                # ScalarE: s~ = sqrt(2(d^2+B)) over w2 first (feeds the big
                # DVE pass), then s = sqrt(d^2+B) over diag LAST so the
                # toucher's single ACT wait covers both PSUM readers.
                nc.scalar.activation(s4[:, :, :, 16:UW], p4[:, :, :, 16:UW],
                                     mybir.ActivationFunctionType.Sqrt,
                                     scale=2.0)
                nc.scalar.activation(s4[:, :, :, 0:16], p4[:, :, :, 0:16],
                                     mybir.ActivationFunctionType.Sqrt)
                # VectorE: v' = min(s~,3sqrt2)-3sqrt2 / min(s,3)-3 (bf16 4x)
                nc.vector.tensor_scalar(
                    v4[:, :, :, 16:UW], s4[:, :, :, 16:UW], T2, T2,
                    mybir.AluOpType.min, mybir.AluOpType.subtract)
                nc.vector.tensor_scalar(
                    v4[:, :, :, 0:16], s4[:, :, :, 0:16], CUTOFF, CUTOFF,
                    mybir.AluOpType.min, mybir.AluOpType.subtract)
                # VectorE: acc = sum v'^2 (single accumulator, 2x mode)
                nc.vector.scalar_tensor_tensor(
                    jq[:], jv[:], 1.0, jv[:],
                    mybir.AluOpType.mult, mybir.AluOpType.mult,
                    accum_out=at[:, 0:1])

                # toucher: retakes PSUM ownership on the PE with one ACT wait
                nc.tensor.matmul(ps[:, 0:1], bw, bx, start=True, stop=True)
        nc.sync.dma_start(acc_d[:], at[:])

    nc.finalize()
    _cache[key] = nc
    return nc


# --------------------------------------------------------------- input prep
def _prepare_inputs(positions, translation, rotation, cell):
    cell64 = cell.astype(np.float64)
    P = _generate(positions, translation, rotation, cell64)      # [N,3] float64
    n = P.shape[0]
    assert n == N, f"kernel hardcodes N={N}, got {n}"

    order = np.argsort(P[:, 2], kind="stable")
    Ps = P[order]
    zs = Ps[:, 2]
    slab_lo = zs.reshape(NCHUNK, 128).min(1)
    slab_hi = zs.reshape(NCHUNK, 128).max(1)

    shifts = np.array([-1.0, 0.0, 1.0])
    offs = np.stack(np.meshgrid(shifts, shifts, shifts, indexing="ij")).reshape(3, -1).T
    vecs = offs @ cell64                                          # [27,3]
    assert np.all(offs[13] == 0.0)

    c = 0.5 * cell64.sum(axis=0)
    reach = CUTOFF + BAND_MARGIN
    lo = P.min(axis=0) - reach
    hi = P.max(axis=0) + reach

    def chunk_cols(S):
        """per-chunk kept image positions for image set S (z-band + box)."""
        keep = np.all((S > lo) & (S < hi), axis=1)
        out = []
        for r in range(NCHUNK):
            m = keep & (S[:, 2] >= slab_lo[r] - reach) & (S[:, 2] <= slab_hi[r] + reach)
            out.append(S[m])
        return out

    # central band columns per chunk (weight 2, cols in higher chunks)
    perch = []
    for r in range(NCHUNK):
        jlo = 128 * (r + 1)
        band = np.nonzero(zs[jlo:] <= slab_hi[r] + reach)[0]
        perch.append([Ps[jlo + band]] if band.size else [])
    loads = np.array([sum(len(x) for x in perch[r]) for r in range(NCHUNK)])

    # greedy mirror choice per half-shift pair to flatten per-chunk loads
    for k in range(13):
        ca = chunk_cols(Ps + vecs[k])
        cb = chunk_cols(Ps + vecs[26 - k])
        la = np.array([len(x) for x in ca])
        lb = np.array([len(x) for x in cb])
        pick = ca if np.max(loads + la) <= np.max(loads + lb) else cb
        lp = la if pick is ca else lb
        loads = loads + lp
        for r in range(NCHUNK):
            if len(pick[r]):
                perch[r].append(pick[r])

    w2_pos = [np.concatenate(perch[r], axis=0) if perch[r] else np.zeros((0, 3))
              for r in range(NCHUNK)]
    wm = int(max(-(-len(p) // NCORES) for p in w2_pos))
    wm = -(-wm // 4) * 4                                          # multiple of 4
    UW = 16 + wm

    dummy_pos = c + 50.0                                          # d^2 >> 9

    in_maps = []
    for core in range(NCORES):
        feat = np.zeros((128, RHS0 + 2 * UW + 64), np.float32)
        for g in range(G):
            rows = slice(32 * g, 32 * g + 5)
            for u in range(2):
                r = 2 * g + u
                feat[rows, 128 * u:128 * (u + 1)] = _featT(
                    Ps[128 * r:128 * (r + 1)], c)
                dcols = Ps[128 * r:128 * (r + 1)][core::NCORES]   # 16 diag
                sel = w2_pos[r][core::NCORES]
                pad = wm - len(sel)
                assert pad >= 0
                if pad:
                    sel = np.concatenate(
                        [sel, np.tile(dummy_pos, (pad, 1))], axis=0)
                cols = np.concatenate([dcols, sel], axis=0)       # [UW,3]
                feat[rows, RHS0 + u * UW:RHS0 + (u + 1) * UW] = _features(
                    cols, c, BIAS)
        in_maps.append({"feat": np.ascontiguousarray(feat)})
    return in_maps, wm


# ------------------------------------------------------------------- runner
def _get_runner(wm, reps: int = 1, dyn_loop: bool = False):
    """Jit the bass program once; reuse the compiled executable per call."""
    key = ("runner", wm, reps, dyn_loop)
    if key in _cache:
        return _cache[key]
    import jax
    from jax.sharding import Mesh, PartitionSpec
    from jax.experimental.shard_map import shard_map
    from concourse import bass2jax, mybir

    nc = _build_program(wm, reps=reps, dyn_loop=dyn_loop)
    bass2jax.install_neuronx_cc_hook()

    partition_name = (
        nc.partition_id_tensor.name if nc.partition_id_tensor else None
    )
    in_names, out_names, out_avals, zero_outs = [], [], [], []
    for alloc in nc.m.functions[0].allocations:
        if not isinstance(alloc, mybir.MemoryLocationSet):
            continue
        name = alloc.memorylocations[0].name
        if alloc.kind == "ExternalInput":
            if name != partition_name:
                in_names.append(name)
        elif alloc.kind == "ExternalOutput":
            out_names.append(name)
            shape = tuple(alloc.tensor_shape)
            dtype = mybir.dt.np(alloc.dtype)
            out_avals.append(jax.core.ShapedArray(shape, dtype))
            zero_outs.append(np.zeros(shape, dtype))
    n_params = len(in_names)
    all_in_names = in_names + out_names
    if partition_name is not None:
        all_in_names = all_in_names + [partition_name]

    def _body(*args):
        operands = list(args)
        if partition_name is not None:
            operands.append(bass2jax.partition_id_tensor())
        outs = bass2jax._bass_exec_p.bind(
            *operands,
            out_avals=tuple(out_avals),
            in_names=tuple(all_in_names),
            out_names=tuple(out_names),
            lowering_input_output_aliases=(),
            sim_require_finite=True,
            sim_require_nnan=True,
            nc=nc,
        )
        return tuple(outs)

    devices = jax.devices()[:NCORES]
    mesh = Mesh(np.asarray(devices), ("core",))
    n_outs = len(out_names)
    sharded = jax.jit(
        shard_map(
            _body, mesh=mesh,
            in_specs=(PartitionSpec("core"),) * (n_params + n_outs),
            out_specs=(PartitionSpec("core"),) * n_outs,
            check_rep=False,
        ),
        keep_unused=True,
    )
    concat_zeros = [
        np.zeros((NCORES * z.shape[0], *z.shape[1:]), z.dtype) for z in zero_outs
    ]

    def run(in_maps):
        concat_in = [
            np.concatenate([in_maps[cc][name] for cc in range(NCORES)], axis=0)
            for name in in_names
        ]
        out_arrs = sharded(*concat_in, *concat_zeros)
        return [
            {
                name: np.asarray(out_arrs[i]).reshape(NCORES, *out_avals[i].shape)[cc]
                for i, name in enumerate(out_names)
            }
            for cc in range(NCORES)
        ]

    _cache[key] = run
    return run


def kernel(positions, translation, rotation, cell, _reps=1, _loop_n=0):
    in_maps, wm = _prepare_inputs(
        np.asarray(positions), np.asarray(translation),
        np.asarray(rotation), np.asarray(cell),
    )
    dyn = _loop_n > 0
    if dyn:
        for m in in_maps:
            m["loopn"] = np.array([[_loop_n]], np.int32)
    run = _get_runner(wm, reps=_reps, dyn_loop=dyn)
    results = run(in_maps)
    total = 0.0
    for r in results:
        total += r["acc"].astype(np.float64).sum()
    # swap device self-pair terms for the exact ones
    total -= N * (CUTOFF - np.sqrt(BIAS)) ** 2
    total += N * (CUTOFF - np.sqrt(np.float32(EPS))) ** 2
    return np.float32(total)


# revision 19
# speedup vs baseline: 3.1911x; 1.4366x over previous
"""Trainium2 Bass kernel for nn_LiquidGenerator.

score = sum over (i, image j) pairs of (CUTOFF - dist)^2 where dist < CUTOFF,
with dist over the [N, 27N] supercell distance matrix.

Strategy (v3)
-------------
Host (O(N log N) prep):
  * generate P (rotation+translation of molecule-local coords, float64)
  * z-sort atoms; rows are processed as 8 chunks of 128 = consecutive z-slabs.
  * central pair symmetry d(i,j)==d(j,i): for row-chunk r only columns j in
    HIGHER chunks are computed (weight 2) plus the full diagonal block
    (weight 1, both orderings).
  * shift symmetry d(i,(k,j)) == d(j,(26-k,i)): one member of each of the 13
    image pairs is computed with weight 2; WHICH member is chosen greedily to
    flatten the per-chunk column loads (the two choices land on mirrored z
    ranges).
  * z-band pruning: a column (central atom or image at z') only pairs with
    chunk r if [z'-3, z'+3] overlaps the chunk's z-slab (~4x fewer elements).
  * distances via the 5-feature inner product
      d^2 + BIAS = [Px,Py,Pz,|P|^2,1] . [-2Sx,-2Sy,-2Sz, 1, |S|^2+BIAS].

Device (8 NeuronCores; every block's columns are sharded core k <- cols k::8):
  per iteration one 4-bank PSUM tile holds 8 uniform units [diag(16)|w2(WM)],
  two per bank: unit = one chunk's diag + weight-2 columns, one self-loading
  fp32 matmul each (8 matmuls, 4-way row-group concurrency).  The weight-2
  factor is folded into the VALUES, not the accumulation:
      sqrt-w2 pass uses scale=2:  s~ = sqrt(2(d^2+B)) = sqrt2 * s
      v' = min(s~, 3*sqrt2) - 3*sqrt2 = sqrt2 * (min(s,3)-3)
  so v'^2 = 2 v^2 and ONE scalar_tensor_tensor square-accumulate over the
  whole tile yields sum(v_diag^2) + 2 sum(v_w2^2) in a single accumulator
  (one DVE accumulator-read per iteration).  All terms are exactly zero for
  non-contributing pairs: no big-sum cancellation, sqrt-spline-safe.
    ScalarE : s~ = sqrt(2(d^2+B)) over w2, s = sqrt(d^2+B) over diag
    VectorE : v' = min(s,3)-3 / min(s~,3sqrt2)-3sqrt2   (bf16, 4x mode)
    VectorE : acc += v'*v' (scalar_tensor_tensor, 2x mode, accum_out)
  score = sum acc - N (3-sqrt(BIAS))^2 + N (3-sqrt(EPS))^2

The timing loop uses a DYNAMIC trip count (read from the `loopn` input) so
one compiled program serves every loop length: the PJRT dispatch constant
cancels exactly in the (wall(N) - wall(1)) / (N-1) slope.  The body holds
`reps` back-to-back iterations so consecutive ones pipeline through the
double-buffered PSUM/SBUF tiles and the all-engine barrier amortizes.
"""

import numpy as np

CUTOFF = 3.0
EPS = 1e-16
BIAS = 2e-4
BAND_MARGIN = 1e-3

NCORES = 8
N = 1024
NCHUNK = 8
G = 4                # PE row groups == PSUM banks
RHS0 = 256           # rhs feature columns start after the two lhs blocks

_cache: dict = {}


# ----------------------------------------------------------------- host math
def _rotation_matrices(rot):
    a, b, g = rot[:, 0], rot[:, 1], rot[:, 2]
    ca, sa = np.cos(a), np.sin(a)
    cb, sb = np.cos(b), np.sin(b)
    cg, sg = np.cos(g), np.sin(g)
    m = rot.shape[0]
    rx = np.zeros((m, 3, 3)); ry = np.zeros((m, 3, 3)); rz = np.zeros((m, 3, 3))
    rx[:, 0, 0] = 1;  rx[:, 1, 1] = ca; rx[:, 1, 2] = -sa; rx[:, 2, 1] = sa; rx[:, 2, 2] = ca
    ry[:, 0, 0] = cb; ry[:, 0, 2] = -sb; ry[:, 1, 1] = 1;  ry[:, 2, 0] = sb; ry[:, 2, 2] = cb
    rz[:, 0, 0] = cg; rz[:, 0, 1] = -sg; rz[:, 1, 0] = sg; rz[:, 1, 1] = cg; rz[:, 2, 2] = 1
    return np.einsum("mij,mjk,mkl->mil", rx, ry, rz)


def _generate(positions, translation, rotation, cell):
    R = _rotation_matrices(rotation.astype(np.float64))
    trans = np.remainder(translation.astype(np.float64), 1.0) @ cell.astype(np.float64)
    gen = np.einsum("mai,mij->maj", positions.astype(np.float64), R) + trans[:, None, :]
    return gen.reshape(-1, 3)


def _features(S, c, bias):
    """rhs feature columns for image positions S (pairs with lhs features)."""
    Sc = (S - c).astype(np.float32)
    return np.stack([
        -2.0 * Sc[:, 0], -2.0 * Sc[:, 1], -2.0 * Sc[:, 2],
        np.ones(S.shape[0], np.float32),
        (Sc.astype(np.float64) ** 2).sum(1).astype(np.float32) + np.float32(bias),
    ]).astype(np.float32)


def _featT(Patoms, c):
    """lhs feature rows [5, n] for row atoms."""
    Pc = (Patoms - c).astype(np.float32)
    return np.stack([
        Pc[:, 0], Pc[:, 1], Pc[:, 2],
        (Pc.astype(np.float64) ** 2).sum(1).astype(np.float32),
        np.ones(Patoms.shape[0], np.float32),
    ]).astype(np.float32)


# ------------------------------------------------------------- bass program
def _build_program(wm: int, reps: int = 1, dyn_loop: bool = False,
                   parts: str = "full"):
    # wm: uniform per-core w2 width of each unit; unit width UW = 16 + wm.
    # parts: "full" | "mm" | "mm+act" | "mm+act+ts" | "noaccum"  (bisection)
    key = ("nc", wm, reps, dyn_loop, parts)
    if key in _cache:
        return _cache[key]
    from contextlib import ExitStack, nullcontext
    import concourse.tile as tile
    from concourse import bacc, mybir

    f32 = mybir.dt.float32
    bf16 = mybir.dt.bfloat16
    i32 = mybir.dt.int32
    UW = 16 + wm
    FD = 2 * G * UW                   # elements per partition per iteration
    FW = RHS0 + 2 * UW + 64           # lhsA lhsB | rhsA rhsB | zero tail
    T2 = float(np.float32(3.0 * np.sqrt(2.0)))

    nc = bacc.Bacc("TRN2", target_bir_lowering=False, debug=False,
                   num_devices=NCORES)
    feat_d = nc.dram_tensor("feat", [128, FW], f32, kind="ExternalInput")
    if dyn_loop:
        loopn_d = nc.dram_tensor("loopn", [1, 1], i32, kind="ExternalInput")
    acc_d = nc.dram_tensor("acc", [128, 1], f32, kind="ExternalOutput")

    with tile.TileContext(nc) as tc, ExitStack() as ctx:
        const = ctx.enter_context(tc.tile_pool(name="const", bufs=1))
        psum = ctx.enter_context(tc.tile_pool(name="psum", bufs=2, space="PSUM"))
        spool = ctx.enter_context(tc.tile_pool(name="s", bufs=2))
        vpool = ctx.enter_context(tc.tile_pool(name="v", bufs=2))
        qpool = ctx.enter_context(tc.tile_pool(name="q", bufs=2))

        ft = const.tile([128, FW], f32)
        nc.sync.dma_start(ft[:], feat_d[:])
        at = const.tile([128, 1], f32)
        if parts != "full":
            nc.vector.memset(at[:], 0.0)   # bisection variants never write it

        # bf16-zero views of the zero-padded feat tail for the toucher matmul
        bw = ft[0:1, FW - 64:FW].bitcast(bf16)  # [1,128]
        bx = bw[:, 0:1]

        if dyn_loop:
            lt = const.tile([1, 1], i32)
            nc.sync.dma_start(lt[:], loopn_d[:])
            nval = nc.values_load(lt[0:1, 0:1], min_val=1, max_val=1 << 30,
                                  skip_runtime_bounds_check=True)
            loop_cm = tc.For_i(0, nval, 1)
        else:
            loop_cm = nullcontext()
        with loop_cm:
            for _u in range(reps):
                ps = psum.tile([128, G * 512], f32)
                for g in range(G):
                    fl = ft[32 * g:32 * g + 5, :]
                    for u in range(2):
                        # one matmul per chunk: [diag(16) | w2(wm)] unit
                        nc.tensor.matmul(
                            ps[:, g * 512 + u * UW:g * 512 + (u + 1) * UW],
                            fl[:, 128 * u:128 * (u + 1)],
                            fl[:, RHS0 + u * UW:RHS0 + (u + 1) * UW],
                            start=True, stop=True, tile_position=(32 * g, 0))

                # views: PSUM [partition, bank, col]; SBUF [partition, unit, col]
                p3 = ps[:].rearrange("p (b w) -> p b w", b=G)[:, :, 0:2 * UW]
                st = spool.tile([128, 2 * G * UW], bf16)
                s3 = st[:].rearrange("p (u w) -> p u w", u=2 * G)
                jv = vpool.tile([128, 2 * G * UW], bf16)
                v3 = jv[:].rearrange("p (u w) -> p u w", u=2 * G)
                jq = qpool.tile([128, 2 * G * UW], bf16)

                # ScalarE: one pass; w2 rhs features are pre-scaled 2x on the
                # host, so PSUM already holds 2(d^2+B) there and
                # s~ = sqrt(.) = sqrt2 * s with a single un-scaled sqrt.
                if parts != "mm":
                    nc.scalar.activation(s3[:], p3[:],
                                         mybir.ActivationFunctionType.Sqrt)
                if parts not in ("mm", "mm+act"):
                    # VectorE: v' = min(s~,3sqrt2)-3sqrt2 / min(s,3)-3 (bf16 4x)
                    nc.vector.tensor_scalar(
                        v3[:, :, 16:UW], s3[:, :, 16:UW], T2, T2,
                        mybir.AluOpType.min, mybir.AluOpType.subtract)
                    nc.vector.tensor_scalar(
                        v3[:, :, 0:16], s3[:, :, 0:16], CUTOFF, CUTOFF,
                        mybir.AluOpType.min, mybir.AluOpType.subtract)
                if parts in ("full", "noaccum"):
                    # VectorE: acc = sum v'^2 (single accumulator, 2x mode)
                    nc.vector.scalar_tensor_tensor(
                        jq[:], jv[:], 1.0, jv[:],
                        mybir.AluOpType.mult, mybir.AluOpType.mult,
                        accum_out=at[:, 0:1] if parts == "full" else None)
                # No PSUM toucher: ACT is the only PSUM reader, so the WAR
                # wait lands on rep u+2's first matmul (a single legal wait)
                # and the PE runs two reps ahead of the ScalarE.
        nc.sync.dma_start(acc_d[:], at[:])

    nc.finalize()
    _cache[key] = nc
    return nc


# --------------------------------------------------------------- input prep
def _prepare_inputs(positions, translation, rotation, cell):
    cell64 = cell.astype(np.float64)
    P = _generate(positions, translation, rotation, cell64)      # [N,3] float64
    n = P.shape[0]
    assert n == N, f"kernel hardcodes N={N}, got {n}"

    order = np.argsort(P[:, 2], kind="stable")
    Ps = P[order]
    zs = Ps[:, 2]
    slab_lo = zs.reshape(NCHUNK, 128).min(1)
    slab_hi = zs.reshape(NCHUNK, 128).max(1)

    shifts = np.array([-1.0, 0.0, 1.0])
    offs = np.stack(np.meshgrid(shifts, shifts, shifts, indexing="ij")).reshape(3, -1).T
    vecs = offs @ cell64                                          # [27,3]
    assert np.all(offs[13] == 0.0)

    c = 0.5 * cell64.sum(axis=0)
    reach = CUTOFF + BAND_MARGIN
    lo = P.min(axis=0) - reach
    hi = P.max(axis=0) + reach

    def chunk_cols(S):
        """per-chunk kept image positions for image set S (z-band + box)."""
        keep = np.all((S > lo) & (S < hi), axis=1)
        out = []
        for r in range(NCHUNK):
            m = keep & (S[:, 2] >= slab_lo[r] - reach) & (S[:, 2] <= slab_hi[r] + reach)
            out.append(S[m])
        return out

    # central band columns per chunk (weight 2, cols in higher chunks)
    perch = []
    for r in range(NCHUNK):
        jlo = 128 * (r + 1)
        band = np.nonzero(zs[jlo:] <= slab_hi[r] + reach)[0]
        perch.append([Ps[jlo + band]] if band.size else [])
    loads = np.array([sum(len(x) for x in perch[r]) for r in range(NCHUNK)])

    # greedy mirror choice per half-shift pair to flatten per-chunk loads
    for k in range(13):
        ca = chunk_cols(Ps + vecs[k])
        cb = chunk_cols(Ps + vecs[26 - k])
        la = np.array([len(x) for x in ca])
        lb = np.array([len(x) for x in cb])
        pick = ca if np.max(loads + la) <= np.max(loads + lb) else cb
        lp = la if pick is ca else lb
        loads = loads + lp
        for r in range(NCHUNK):
            if len(pick[r]):
                perch[r].append(pick[r])

    w2_pos = [np.concatenate(perch[r], axis=0) if perch[r] else np.zeros((0, 3))
              for r in range(NCHUNK)]
    wm = int(max(-(-len(p) // NCORES) for p in w2_pos))
    wm = -(-wm // 4) * 4                                          # multiple of 4
    UW = 16 + wm

    dummy_pos = c + 50.0                                          # d^2 >> 9

    in_maps = []
    for core in range(NCORES):
        feat = np.zeros((128, RHS0 + 2 * UW + 64), np.float32)
        for g in range(G):
            rows = slice(32 * g, 32 * g + 5)
            for u in range(2):
                r = 2 * g + u
                feat[rows, 128 * u:128 * (u + 1)] = _featT(
                    Ps[128 * r:128 * (r + 1)], c)
                dcols = Ps[128 * r:128 * (r + 1)][core::NCORES]   # 16 diag
                sel = w2_pos[r][core::NCORES]
                pad = wm - len(sel)
                assert pad >= 0
                if pad:
                    sel = np.concatenate(
                        [sel, np.tile(dummy_pos, (pad, 1))], axis=0)
                off = RHS0 + u * UW
                feat[rows, off:off + 16] = _features(dcols, c, BIAS)
                # w2 rhs features pre-scaled 2x (exact): PSUM gets 2(d^2+B)
                feat[rows, off + 16:off + UW] = 2.0 * _features(sel, c, BIAS)
        in_maps.append({"feat": np.ascontiguousarray(feat)})
    return in_maps, wm


# ------------------------------------------------------------------- runner
def _get_runner(wm, reps: int = 1, dyn_loop: bool = False, parts: str = "full"):
    """Jit the bass program once; reuse the compiled executable per call."""
    key = ("runner", wm, reps, dyn_loop, parts)
    if key in _cache:
        return _cache[key]
    import jax
    from jax.sharding import Mesh, PartitionSpec
    from jax.experimental.shard_map import shard_map
    from concourse import bass2jax, mybir

    nc = _build_program(wm, reps=reps, dyn_loop=dyn_loop, parts=parts)
    bass2jax.install_neuronx_cc_hook()

    partition_name = (
        nc.partition_id_tensor.name if nc.partition_id_tensor else None
    )
    in_names, out_names, out_avals, zero_outs = [], [], [], []
    for alloc in nc.m.functions[0].allocations:
        if not isinstance(alloc, mybir.MemoryLocationSet):
            continue
        name = alloc.memorylocations[0].name
        if alloc.kind == "ExternalInput":
            if name != partition_name:
                in_names.append(name)
        elif alloc.kind == "ExternalOutput":
            out_names.append(name)
            shape = tuple(alloc.tensor_shape)
            dtype = mybir.dt.np(alloc.dtype)
            out_avals.append(jax.core.ShapedArray(shape, dtype))
            zero_outs.append(np.zeros(shape, dtype))
    n_params = len(in_names)
    all_in_names = in_names + out_names
    if partition_name is not None:
        all_in_names = all_in_names + [partition_name]

    def _body(*args):
        operands = list(args)
        if partition_name is not None:
            operands.append(bass2jax.partition_id_tensor())
        outs = bass2jax._bass_exec_p.bind(
            *operands,
            out_avals=tuple(out_avals),
            in_names=tuple(all_in_names),
            out_names=tuple(out_names),
            lowering_input_output_aliases=(),
            sim_require_finite=True,
            sim_require_nnan=True,
            nc=nc,
        )
        return tuple(outs)

    devices = jax.devices()[:NCORES]
    mesh = Mesh(np.asarray(devices), ("core",))
    n_outs = len(out_names)
    sharded = jax.jit(
        shard_map(
            _body, mesh=mesh,
            in_specs=(PartitionSpec("core"),) * (n_params + n_outs),
            out_specs=(PartitionSpec("core"),) * n_outs,
            check_rep=False,
        ),
        keep_unused=True,
    )
    concat_zeros = [
        np.zeros((NCORES * z.shape[0], *z.shape[1:]), z.dtype) for z in zero_outs
    ]

    def run(in_maps):
        concat_in = [
            np.concatenate([in_maps[cc][name] for cc in range(NCORES)], axis=0)
            for name in in_names
        ]
        out_arrs = sharded(*concat_in, *concat_zeros)
        return [
            {
                name: np.asarray(out_arrs[i]).reshape(NCORES, *out_avals[i].shape)[cc]
                for i, name in enumerate(out_names)
            }
            for cc in range(NCORES)
        ]

    _cache[key] = run
    return run


def kernel(positions, translation, rotation, cell, _reps=1, _loop_n=0,
           _parts="full"):
    in_maps, wm = _prepare_inputs(
        np.asarray(positions), np.asarray(translation),
        np.asarray(rotation), np.asarray(cell),
    )
    dyn = _loop_n > 0
    if dyn:
        for m in in_maps:
            m["loopn"] = np.array([[_loop_n]], np.int32)
    run = _get_runner(wm, reps=_reps, dyn_loop=dyn, parts=_parts)
    results = run(in_maps)
    total = 0.0
    for r in results:
        total += r["acc"].astype(np.float64).sum()
    # swap device self-pair terms for the exact ones
    total -= N * (CUTOFF - np.sqrt(BIAS)) ** 2
    total += N * (CUTOFF - np.sqrt(np.float32(EPS))) ** 2
    return np.float32(total)


# revision 34
# speedup vs baseline: 4.5882x; 1.4378x over previous
"""Trainium2 Bass kernel for nn_LiquidGenerator.

score = sum over (i, image j) pairs of (CUTOFF - dist)^2 where dist < CUTOFF,
with dist over the [N, 27N] supercell distance matrix.

Strategy (v3)
-------------
Host (O(N log N) prep):
  * generate P (rotation+translation of molecule-local coords, float64)
  * z-sort atoms; rows are processed as 8 chunks of 128 = consecutive z-slabs.
  * central pair symmetry d(i,j)==d(j,i): for row-chunk r only columns j in
    HIGHER chunks are computed (weight 2) plus the full diagonal block
    (weight 1, both orderings).
  * shift symmetry d(i,(k,j)) == d(j,(26-k,i)): one member of each of the 13
    image pairs is computed with weight 2; WHICH member is chosen greedily to
    flatten the per-chunk column loads (the two choices land on mirrored z
    ranges).
  * z-band pruning: a column (central atom or image at z') only pairs with
    chunk r if [z'-3, z'+3] overlaps the chunk's z-slab (~4x fewer elements).
  * distances via the 5-feature inner product
      d^2 + BIAS = [Px,Py,Pz,|P|^2,1] . [-2Sx,-2Sy,-2Sz, 1, |S|^2+BIAS].

Device (8 NeuronCores; every block's columns are sharded core k <- cols k::8):
  per iteration one 4-bank PSUM tile holds 8 uniform units [diag(16)|w2(WM)],
  two per bank: unit = one chunk's diag + weight-2 columns, one self-loading
  fp32 matmul each (8 matmuls, 4-way row-group concurrency).  The weight-2
  factor is folded into the VALUES, not the accumulation:
      sqrt-w2 pass uses scale=2:  s~ = sqrt(2(d^2+B)) = sqrt2 * s
      v' = min(s~, 3*sqrt2) - 3*sqrt2 = sqrt2 * (min(s,3)-3)
  so v'^2 = 2 v^2 and ONE scalar_tensor_tensor square-accumulate over the
  whole tile yields sum(v_diag^2) + 2 sum(v_w2^2) in a single accumulator
  (one DVE accumulator-read per iteration).  All terms are exactly zero for
  non-contributing pairs: no big-sum cancellation, sqrt-spline-safe.
    ScalarE : s~ = sqrt(2(d^2+B)) over w2, s = sqrt(d^2+B) over diag
    VectorE : v' = min(s,3)-3 / min(s~,3sqrt2)-3sqrt2   (bf16, 4x mode)
    VectorE : acc += v'*v' (scalar_tensor_tensor, 2x mode, accum_out)
  score = sum acc - N (3-sqrt(BIAS))^2 + N (3-sqrt(EPS))^2

The timing loop uses a DYNAMIC trip count (read from the `loopn` input) so
one compiled program serves every loop length: the PJRT dispatch constant
cancels exactly in the (wall(N) - wall(1)) / (N-1) slope.  The body holds
`reps` back-to-back iterations so consecutive ones pipeline through the
double-buffered PSUM/SBUF tiles and the all-engine barrier amortizes.
"""

import numpy as np

CUTOFF = 3.0
EPS = 1e-16
BIAS = 2e-4
BAND_MARGIN = 1e-3

NCORES = 8
N = 1024
NCHUNK = 8
G = 4                # PE row groups == PSUM banks
RHS0 = 256           # rhs feature columns start after the two lhs blocks

_cache: dict = {}


# ----------------------------------------------------------------- host math
def _rotation_matrices(rot):
    a, b, g = rot[:, 0], rot[:, 1], rot[:, 2]
    ca, sa = np.cos(a), np.sin(a)
    cb, sb = np.cos(b), np.sin(b)
    cg, sg = np.cos(g), np.sin(g)
    m = rot.shape[0]
    rx = np.zeros((m, 3, 3)); ry = np.zeros((m, 3, 3)); rz = np.zeros((m, 3, 3))
    rx[:, 0, 0] = 1;  rx[:, 1, 1] = ca; rx[:, 1, 2] = -sa; rx[:, 2, 1] = sa; rx[:, 2, 2] = ca
    ry[:, 0, 0] = cb; ry[:, 0, 2] = -sb; ry[:, 1, 1] = 1;  ry[:, 2, 0] = sb; ry[:, 2, 2] = cb
    rz[:, 0, 0] = cg; rz[:, 0, 1] = -sg; rz[:, 1, 0] = sg; rz[:, 1, 1] = cg; rz[:, 2, 2] = 1
    return np.einsum("mij,mjk,mkl->mil", rx, ry, rz)


def _generate(positions, translation, rotation, cell):
    R = _rotation_matrices(rotation.astype(np.float64))
    trans = np.remainder(translation.astype(np.float64), 1.0) @ cell.astype(np.float64)
    gen = np.einsum("mai,mij->maj", positions.astype(np.float64), R) + trans[:, None, :]
    return gen.reshape(-1, 3)


def _features(S, c, bias):
    """rhs feature columns for image positions S (pairs with lhs features)."""
    Sc = (S - c).astype(np.float32)
    return np.stack([
        -2.0 * Sc[:, 0], -2.0 * Sc[:, 1], -2.0 * Sc[:, 2],
        np.ones(S.shape[0], np.float32),
        (Sc.astype(np.float64) ** 2).sum(1).astype(np.float32) + np.float32(bias),
    ]).astype(np.float32)


def _featT(Patoms, c):
    """lhs feature rows [5, n] for row atoms."""
    Pc = (Patoms - c).astype(np.float32)
    return np.stack([
        Pc[:, 0], Pc[:, 1], Pc[:, 2],
        (Pc.astype(np.float64) ** 2).sum(1).astype(np.float32),
        np.ones(Patoms.shape[0], np.float32),
    ]).astype(np.float32)


# ------------------------------------------------------------- bass program
def _build_program(w2b: int, reps: int = 1, dyn_loop: bool = False,
                   parts: str = "full"):
    # w2b: per-core padded weight-2 width per bank (2 chunks' w2 cols).
    # parts: "full" | "mm" | "mm+act" | "mm+act+ts" | "noaccum"  (bisection)
    key = ("nc", w2b, reps, dyn_loop, parts)
    if key in _cache:
        return _cache[key]
    from contextlib import ExitStack, nullcontext
    import concourse.tile as tile
    from concourse import bacc, mybir

    f32 = mybir.dt.float32
    bf16 = mybir.dt.bfloat16
    i32 = mybir.dt.int32
    BW = 32 + w2b                     # live PSUM cols per bank
    W2A = G * w2b                     # total w2 cols (s-tile region size)
    FD = G * BW                       # elements per partition per iteration
    # lhs: 2 chunks' features K-stacked per row group (K=10, rows 32g..+10);
    # rhs: one [diagA diagB w2AB] block per group; zero rows kill
    # cross-chunk terms.
    FW = 128 + BW + 64
    T2 = float(np.float32(3.0 * np.sqrt(2.0)))

    nc = bacc.Bacc("TRN2", target_bir_lowering=False, debug=False,
                   num_devices=NCORES)
    feat_d = nc.dram_tensor("feat", [128, FW], f32, kind="ExternalInput")
    if dyn_loop:
        loopn_d = nc.dram_tensor("loopn", [1, 1], i32, kind="ExternalInput")
    acc_d = nc.dram_tensor("acc", [128, 1], f32, kind="ExternalOutput")

    with tile.TileContext(nc) as tc, ExitStack() as ctx:
        const = ctx.enter_context(tc.tile_pool(name="const", bufs=1))
        psum = ctx.enter_context(tc.tile_pool(name="psum", bufs=2, space="PSUM"))
        spool = ctx.enter_context(tc.tile_pool(name="s", bufs=2))
        vpool = ctx.enter_context(tc.tile_pool(name="v", bufs=2))
        qpool = ctx.enter_context(tc.tile_pool(name="q", bufs=2))

        ft = const.tile([128, FW], f32)
        nc.sync.dma_start(ft[:], feat_d[:])
        at = const.tile([128, 1], f32)
        if parts != "full":
            nc.vector.memset(at[:], 0.0)   # bisection variants never write it

        # bf16-zero views of the zero-padded feat tail for the toucher matmul
        bw = ft[0:1, FW - 64:FW].bitcast(bf16)  # [1,128]
        bx = bw[:, 0:1]

        if dyn_loop:
            lt = const.tile([1, 1], i32)
            nc.sync.dma_start(lt[:], loopn_d[:])
            nval = nc.values_load(lt[0:1, 0:1], min_val=1, max_val=1 << 30,
                                  skip_runtime_bounds_check=True)
            loop_cm = tc.For_i(0, nval, 1)
        else:
            loop_cm = nullcontext()
        with loop_cm:
            for _u in range(reps):
                ps = psum.tile([128, G * 512], f32)
                for g in range(G):
                    # 2 chunks K-stacked (K=10): one matmul covers both
                    # chunks' [diagA diagB w2AB] block; each column's rhs
                    # rows outside its own chunk's 5 features are zero, so
                    # cross-chunk terms vanish exactly.
                    fl = ft[32 * g:32 * g + 10, :]
                    nc.tensor.matmul(
                        ps[:, g * 512:g * 512 + BW],
                        fl[:, 0:128],
                        fl[:, 128:128 + BW],
                        start=True, stop=True, tile_position=(32 * g, 0))

                # s-tile layout: [w2-all (G*w2b) | diag-all (128)], both
                # written packed so the DVE passes read flat 2D ranges.
                pb = ps[:].rearrange("p (b w) -> p b w", b=G)
                st = spool.tile([128, W2A + 128], bf16)
                jv = vpool.tile([128, W2A + 128], bf16)
                jq = qpool.tile([128, W2A + 128], bf16)
                sw = st[:, 0:W2A].rearrange("p (b w) -> p b w", b=G)
                sd = st[:, W2A:W2A + 128].rearrange("p (b w) -> p b w", b=G)

                # ScalarE: w2 sqrt first (feeds the big DVE pass), diag last;
                # w2 rhs features are pre-scaled 2x on the host, so PSUM
                # already holds 2(d^2+B) there: s~ = sqrt2 * s, no scale op.
                if parts != "mm":
                    nc.scalar.activation(sw, pb[:, :, 32:BW],
                                         mybir.ActivationFunctionType.Sqrt)
                    nc.scalar.activation(sd, pb[:, :, 0:32],
                                         mybir.ActivationFunctionType.Sqrt)
                if parts not in ("mm", "mm+act"):
                    # VectorE: v' = min(s~,3sqrt2)-3sqrt2 / min(s,3)-3 (bf16 4x)
                    nc.vector.tensor_scalar(
                        jv[:, 0:W2A], st[:, 0:W2A], T2, T2,
                        mybir.AluOpType.min, mybir.AluOpType.subtract)
                    nc.vector.tensor_scalar(
                        jv[:, W2A:W2A + 128], st[:, W2A:W2A + 128],
                        CUTOFF, CUTOFF,
                        mybir.AluOpType.min, mybir.AluOpType.subtract)
                if parts in ("full", "noaccum"):
                    # VectorE: acc = sum v'^2 (single accumulator, 2x mode)
                    nc.vector.scalar_tensor_tensor(
                        jq[:], jv[:], 1.0, jv[:],
                        mybir.AluOpType.mult, mybir.AluOpType.mult,
                        accum_out=at[:, 0:1] if parts == "full" else None)
                # No PSUM toucher: ACT is the only PSUM reader, so the WAR
                # wait lands on rep u+2's first matmul (a single legal wait)
                # and the PE runs two reps ahead of the ScalarE.
        nc.sync.dma_start(acc_d[:], at[:])

    nc.finalize()
    _cache[key] = nc
    return nc


# --------------------------------------------------------------- input prep
def _prepare_inputs(positions, translation, rotation, cell):
    cell64 = cell.astype(np.float64)
    P = _generate(positions, translation, rotation, cell64)      # [N,3] float64
    n = P.shape[0]
    assert n == N, f"kernel hardcodes N={N}, got {n}"

    order = np.argsort(P[:, 2], kind="stable")
    Ps = P[order]
    zs = Ps[:, 2]
    slab_lo = zs.reshape(NCHUNK, 128).min(1)
    slab_hi = zs.reshape(NCHUNK, 128).max(1)

    shifts = np.array([-1.0, 0.0, 1.0])
    offs = np.stack(np.meshgrid(shifts, shifts, shifts, indexing="ij")).reshape(3, -1).T
    vecs = offs @ cell64                                          # [27,3]
    assert np.all(offs[13] == 0.0)

    c = 0.5 * cell64.sum(axis=0)
    reach = CUTOFF + BAND_MARGIN
    lo = P.min(axis=0) - reach
    hi = P.max(axis=0) + reach

    def chunk_cols(S):
        """per-chunk kept image positions for image set S (z-band + box)."""
        keep = np.all((S > lo) & (S < hi), axis=1)
        out = []
        for r in range(NCHUNK):
            m = keep & (S[:, 2] >= slab_lo[r] - reach) & (S[:, 2] <= slab_hi[r] + reach)
            out.append(S[m])
        return out

    # Two kinds of two-sided choices, greedily assigned to flatten the
    # padded per-bank width = max big-with-small pair sum of chunk loads:
    #  * central chunk-pair (r,q): the weight-2 block can sit at rows r
    #    (cols = q's atoms within reach) or rows q (cols = r's atoms)
    #  * half-shift pair (k, 26-k): mirrored image columns land on
    #    mirrored z ranges
    def pair_cost(ld):
        s = np.sort(ld)
        return int(np.max(s + s[::-1]))

    items = []
    for r in range(NCHUNK):
        for q in range(r + 1, NCHUNK):
            a = Ps[128 * q:128 * (q + 1)]
            a = a[a[:, 2] <= slab_hi[r] + reach]
            b = Ps[128 * r:128 * (r + 1)]
            b = b[b[:, 2] >= slab_lo[q] - reach]
            if len(a) == 0 and len(b) == 0:
                continue
            la = np.zeros(NCHUNK, int); la[r] = len(a)
            lb = np.zeros(NCHUNK, int); lb[q] = len(b)
            items.append((la, {r: a} if len(a) else {},
                          lb, {q: b} if len(b) else {}))
    for k in range(13):
        ca = chunk_cols(Ps + vecs[k])
        cb = chunk_cols(Ps + vecs[26 - k])
        items.append((np.array([len(x) for x in ca]),
                      {r: ca[r] for r in range(NCHUNK) if len(ca[r])},
                      np.array([len(x) for x in cb]),
                      {r: cb[r] for r in range(NCHUNK) if len(cb[r])}))

    perch = [[] for _ in range(NCHUNK)]
    loads = np.zeros(NCHUNK, int)
    items.sort(key=lambda it: -max(it[0].sum(), it[2].sum()))
    for la, da, lb, db in items:
        if pair_cost(loads + la) <= pair_cost(loads + lb):
            lp, dp = la, da
        else:
            lp, dp = lb, db
        loads = loads + lp
        for r, cols in dp.items():
            perch[r].append(cols)

    w2_pos = [np.concatenate(perch[r], axis=0) if perch[r] else np.zeros((0, 3))
              for r in range(NCHUNK)]
    # per-core widths; pair big-with-small chunks per bank to minimize the
    # padded per-bank w2 width
    pw = np.array([-(-len(p) // NCORES) for p in w2_pos])
    idx = np.argsort(pw, kind="stable")[::-1]
    pairs = [(int(idx[b]), int(idx[7 - b])) for b in range(G)]
    w2b = int(max(pw[a] + pw[b] for a, b in pairs))
    w2b = -(-w2b // 4) * 4                                        # multiple of 4
    BW = 32 + w2b

    dummy_pos = c + 50.0                                          # d^2 >> 9

    in_maps = []
    for core in range(NCORES):
        feat = np.zeros((128, 128 + BW + 64), np.float32)
        for g, (ra, rb) in enumerate(pairs):
            # row group g: chunk ra in rows 32g..+5, rb in rows 32g+5..+10;
            # rhs block [diagA(16) diagB(16) w2A w2B pad]; each column only
            # fills its own chunk's 5 feature rows.
            base = 32 * g
            for u, r in enumerate((ra, rb)):
                rows = slice(base + 5 * u, base + 5 * u + 5)
                feat[rows, 0:128] = _featT(Ps[128 * r:128 * (r + 1)], c)
                dcols = Ps[128 * r:128 * (r + 1)][core::NCORES]   # 16 diag
                feat[rows, 128 + 16 * u:128 + 16 * u + 16] = _features(
                    dcols, c, BIAS)
            wa = int(pw[ra])
            sela = w2_pos[ra][core::NCORES]
            selb = w2_pos[rb][core::NCORES]
            pad = w2b - wa - len(selb)
            assert pad >= 0 and len(sela) <= wa
            pada = wa - len(sela)
            if pada:
                sela = np.concatenate(
                    [sela, np.tile(dummy_pos, (pada, 1))], axis=0)
            if pad:
                selb = np.concatenate(
                    [selb, np.tile(dummy_pos, (pad, 1))], axis=0)
            # w2 rhs features pre-scaled 2x (exact): PSUM gets 2(d^2+B)
            feat[slice(base, base + 5), 128 + 32:128 + 32 + wa] = \
                2.0 * _features(sela, c, BIAS)
            feat[slice(base + 5, base + 10), 128 + 32 + wa:128 + BW] = \
                2.0 * _features(selb, c, BIAS)
        in_maps.append({"feat": np.ascontiguousarray(feat)})
    return in_maps, w2b


# ------------------------------------------------------------------- runner
def _get_runner(wm, reps: int = 1, dyn_loop: bool = False, parts: str = "full"):
    """Jit the bass program once; reuse the compiled executable per call."""
    key = ("runner", wm, reps, dyn_loop, parts)
    if key in _cache:
        return _cache[key]
    import jax
    from jax.sharding import Mesh, PartitionSpec
    from jax.experimental.shard_map import shard_map
    from concourse import bass2jax, mybir

    nc = _build_program(wm, reps=reps, dyn_loop=dyn_loop, parts=parts)
    bass2jax.install_neuronx_cc_hook()

    partition_name = (
        nc.partition_id_tensor.name if nc.partition_id_tensor else None
    )
    in_names, out_names, out_avals, zero_outs = [], [], [], []
    for alloc in nc.m.functions[0].allocations:
        if not isinstance(alloc, mybir.MemoryLocationSet):
            continue
        name = alloc.memorylocations[0].name
        if alloc.kind == "ExternalInput":
            if name != partition_name:
                in_names.append(name)
        elif alloc.kind == "ExternalOutput":
            out_names.append(name)
            shape = tuple(alloc.tensor_shape)
            dtype = mybir.dt.np(alloc.dtype)
            out_avals.append(jax.core.ShapedArray(shape, dtype))
            zero_outs.append(np.zeros(shape, dtype))
    n_params = len(in_names)
    all_in_names = in_names + out_names
    if partition_name is not None:
        all_in_names = all_in_names + [partition_name]

    def _body(*args):
        operands = list(args)
        if partition_name is not None:
            operands.append(bass2jax.partition_id_tensor())
        outs = bass2jax._bass_exec_p.bind(
            *operands,
            out_avals=tuple(out_avals),
            in_names=tuple(all_in_names),
            out_names=tuple(out_names),
            lowering_input_output_aliases=(),
            sim_require_finite=True,
            sim_require_nnan=True,
            nc=nc,
        )
        return tuple(outs)

    devices = jax.devices()[:NCORES]
    mesh = Mesh(np.asarray(devices), ("core",))
    n_outs = len(out_names)
    sharded = jax.jit(
        shard_map(
            _body, mesh=mesh,
            in_specs=(PartitionSpec("core"),) * (n_params + n_outs),
            out_specs=(PartitionSpec("core"),) * n_outs,
            check_rep=False,
        ),
        keep_unused=True,
    )
    concat_zeros = [
        np.zeros((NCORES * z.shape[0], *z.shape[1:]), z.dtype) for z in zero_outs
    ]

    def run(in_maps):
        concat_in = [
            np.concatenate([in_maps[cc][name] for cc in range(NCORES)], axis=0)
            for name in in_names
        ]
        out_arrs = sharded(*concat_in, *concat_zeros)
        return [
            {
                name: np.asarray(out_arrs[i]).reshape(NCORES, *out_avals[i].shape)[cc]
                for i, name in enumerate(out_names)
            }
            for cc in range(NCORES)
        ]

    _cache[key] = run
    return run


def kernel(positions, translation, rotation, cell, _reps=1, _loop_n=0,
           _parts="full"):
    in_maps, wm = _prepare_inputs(
        np.asarray(positions), np.asarray(translation),
        np.asarray(rotation), np.asarray(cell),
    )
    dyn = _loop_n > 0
    if dyn:
        for m in in_maps:
            m["loopn"] = np.array([[_loop_n]], np.int32)
    run = _get_runner(wm, reps=_reps, dyn_loop=dyn, parts=_parts)
    results = run(in_maps)
    total = 0.0
    for r in results:
        total += r["acc"].astype(np.float64).sum()
    # swap device self-pair terms for the exact ones
    total -= N * (CUTOFF - np.sqrt(BIAS)) ** 2
    total += N * (CUTOFF - np.sqrt(np.float32(EPS))) ** 2
    return np.float32(total)
